# revision 14
# baseline (speedup 1.0000x reference)
"""Multi-head causal attention (B=8, S=1024, D=768, H=12) on 8 trn2 NeuronCores.

Strategy: data-parallel over batch (one batch element per core, no collectives).

v2 design (all-bf16 operands, fp32 PSUM accumulation):
  - bf16 x/W halve input DMA (6MB vs 16MB) and enable PE fast-weight-load.
  - causal blocking at (128 kpos x 256 q) granularity: 0.625 of S^2 computed
    (vs 0.75 at 512-q blocks), scores as S^T[k,q] = K @ Q^T per head, two
    heads packed into the 128-row PE array via tile_position row tiling.
  - softmax: exp on ScalarE from PSUM [128, 2, 256] tiles; 1/8 scale folded
    into W_q host-side; 0/1 bf16 mask multiply (DVE) only on the two
    diagonal-crossing kpos blocks of each q panel.
  - A@V with ones column appended to V (denominator rides row 64 of the
    [65, 512] PSUM); division folded into the PSUM->SBUF copy (fast
    reciprocal + DMA partition-broadcast through a DRAM scratch).
  - phase schedule keeps the PE dense (HAM warm): PRE emits V-proj st0..3 +
    Q/K proj for q columns 0:512; attention runs q panels ascending
    (qc0: qw0,qw1 then qc1: qw2,qw3) with remaining V-proj/Q/K-proj/output-
    projection work woven between attention steps; y rows stream out as
    their q panel completes.
"""
import sys

if "/opt/trn_rl_repo" not in sys.path:
    sys.path.insert(0, "/opt/trn_rl_repo")

import numpy as np

B, S, D, H = 8, 1024, 768, 12
DH = 64
NC_ = 8
NT = D // 128     # 6
ST = S // 128     # 8
QW = S // 256     # 4 q panels
QC = S // 512     # 2

_compiled = None


def _build_mask():
    import ml_dtypes

    p = np.arange(128)[:, None, None, None]
    j = np.arange(2)[None, :, None, None]
    hh = np.arange(2)[None, None, :, None]
    c = np.arange(256)[None, None, None, :]
    m = (c >= 128 * j + p).astype(np.float32) + 0 * hh
    return np.broadcast_to(m, (128, 2, 2, 256)).astype(ml_dtypes.bfloat16)


def _build_nc():
    import concourse.bass as bass
    import concourse.mybir as mybir
    import concourse.tile as tile
    from concourse import bacc

    F32 = mybir.dt.float32
    BF16 = mybir.dt.bfloat16
    AF = mybir.ActivationFunctionType
    MULT = mybir.AluOpType.mult

    nc = bacc.Bacc("TRN2", target_bir_lowering=False, debug=False)

    xT_d = nc.dram_tensor("xT", [128, NT, S], BF16, kind="ExternalInput")
    wq_d = nc.dram_tensor("wq", [128, NT, D], BF16, kind="ExternalInput")
    wk_d = nc.dram_tensor("wk", [128, NT, D], BF16, kind="ExternalInput")
    wv_d = nc.dram_tensor("wv", [128, NT, D], BF16, kind="ExternalInput")
    wp_d = nc.dram_tensor("wp", [128, NT, D], BF16, kind="ExternalInput")
    mask_d = nc.dram_tensor("masks", [128, 2, 2, 256], BF16, kind="ExternalInput")
    y_d = nc.dram_tensor("y", [S, D], BF16, kind="ExternalOutput")
    recip_d = nc.dram_tensor("recip_scratch", [H, QC, 512], F32)

    with tile.TileContext(nc) as tc:
        with (
            tc.tile_pool(name="static", bufs=1) as static,
            tc.tile_pool(name="pt", bufs=5) as ptp,
            tc.tile_pool(name="mh", bufs=3) as mhp,
            tc.tile_pool(name="small", bufs=4) as smallp,
            tc.tile_pool(name="rb", bufs=3) as rbp,
            tc.tile_pool(name="y", bufs=2) as ypool,
            tc.tile_pool(name="pss", bufs=2, space="PSUM") as pss,
            tc.tile_pool(name="pav", bufs=3, space="PSUM") as pav,
            tc.tile_pool(name="pproj", bufs=1, space="PSUM") as pproj,
        ):
            # ---- persistent SBUF ----
            xT = static.tile([128, NT, S], BF16)
            qT = static.tile([128, NT, S], BF16)
            kT = static.tile([128, NT, S], BF16)
            outT = static.tile([128, NT, S], BF16)
            vp = static.tile([128, ST, H * (DH + 1)], BF16)
            msk = static.tile([128, 2, 2, 256], BF16)
            wq_sb = static.tile([128, NT, D], BF16)
            wk_sb = static.tile([128, NT, D], BF16)
            wv_sb = static.tile([128, NT, D], BF16)
            wp_sb = static.tile([128, NT, D], BF16)

            # input DMAs split across the two hwdge queues (sync + scalar)
            for dc in (0, 2, 4):
                nc.sync.dma_start(xT[:, dc, :], xT_d[:, dc, :])
            for dc in (1, 3, 5):
                nc.scalar.dma_start(xT[:, dc, :], xT_d[:, dc, :])
            nc.sync.dma_start(msk[:], mask_d[:])
            nc.scalar.dma_start(wk_sb[:], wk_d[:])
            nc.sync.dma_start(wq_sb[:], wq_d[:])
            nc.scalar.dma_start(wv_sb[:], wv_d[:])
            nc.sync.dma_start(wp_sb[:], wp_d[:])
            nc.vector.memset(vp[:], 1.0)

            # ---- work units, emitted as ~2-matmul chunks ----
            # pool="pss": [128, 2, 512] 2-bank tile, halves A=[:,0,:] (512)
            #   B=[:,1,0:256] — used in PRE/tail when scores are idle.
            # pool="pproj": single-bank [128, 512] tiles — used for fillers
            #   woven into attention (A and B are separate staggered units).
            uid = [0]

            def v_unit_chunks(st, pool, halves=(0, 1)):
                state = {}

                def mk(half, dcs, fin):
                    def chunk():
                        key = f"v{st}_{half}"
                        if key not in state:
                            if pool == "pss":
                                if "t" not in state:
                                    state["t"] = pss.tile([128, 2, 512], F32, tag="ps", name=f"vps{st}")
                                state[key] = state["t"][:, half, 0:512 if half == 0 else 256]
                            else:
                                t = pproj.tile([128, 512], F32, tag="proj", name=key)
                                state[key] = t[:, 0:512 if half == 0 else 256]
                        ps = state[key]
                        w = 512 if half == 0 else 256
                        for dc in dcs:
                            nc.tensor.matmul(
                                ps[:, 0:w], xT[:, dc, 128 * st:128 * (st + 1)],
                                wv_sb[:, dc, 512 * half:512 * half + w],
                                start=(dc == 0), stop=(dc == NT - 1))
                        if fin:
                            dst = vp[:, st, :].rearrange("p (h e) -> p h e", e=DH + 1)
                            if half == 0:
                                nc.vector.tensor_copy(
                                    out=dst[:, 0:8, 0:DH],
                                    in_=ps[:, 0:512].rearrange("p (h d) -> p h d", d=DH))
                            else:
                                nc.vector.tensor_copy(
                                    out=dst[:, 8:12, 0:DH],
                                    in_=ps[:, 0:256].rearrange("p (h d) -> p h d", d=DH))
                    return chunk

                out = []
                for half in halves:
                    out.append(mk(half, (0, 1), False))
                    out.append(mk(half, (2, 3), False))
                    out.append(mk(half, (4, 5), True))
                return out

            def proj_unit_chunks(w_sb, dst, nt, sc, pool):
                state = {}

                def mk(dcs, fin):
                    def chunk():
                        if "ps" not in state:
                            uid[0] += 1
                            if pool == "pss":
                                t = pss.tile([128, 2, 512], F32, tag="ps", name=f"qk{uid[0]}")
                                state["ps"] = t[:, 0, :]
                            else:
                                state["ps"] = pproj.tile([128, 512], F32, tag="proj", name=f"qk{uid[0]}")
                        ps = state["ps"]
                        for dc in dcs:
                            nc.tensor.matmul(
                                ps[:],
                                w_sb[:, dc, 128 * nt:128 * (nt + 1)],
                                xT[:, dc, 512 * sc:512 * (sc + 1)],
                                start=(dc == 0), stop=(dc == NT - 1))
                        if fin:
                            nc.vector.tensor_copy(
                                out=dst[:, nt, 512 * sc:512 * (sc + 1)], in_=ps[:])
                    return chunk

                return [mk((0, 1), False), mk((2, 3), False), mk((4, 5), True)]

            def ste_unit_chunks(st, pool, halves=(0, 1)):
                state = {}

                def mk(half, dcs, fin):
                    def chunk():
                        key = f"e{st}_{half}"
                        if key not in state:
                            if pool == "pss":
                                if "t" not in state:
                                    state["t"] = pss.tile([128, 2, 512], F32, tag="ps", name=f"eps{st}")
                                state[key] = state["t"][:, half, 0:512 if half == 0 else 256]
                            else:
                                t = pproj.tile([128, 512], F32, tag="proj", name=key)
                                state[key] = t[:, 0:512 if half == 0 else 256]
                        ps = state[key]
                        w = 512 if half == 0 else 256
                        for dc in dcs:
                            nc.tensor.matmul(
                                ps[:, 0:w], outT[:, dc, 128 * st:128 * (st + 1)],
                                wp_sb[:, dc, 512 * half:512 * half + w],
                                start=(dc == 0), stop=(dc == NT - 1))
                        if fin:
                            key_y = f"y{st}"
                            if key_y not in state:
                                state[key_y] = ypool.tile([128, D], BF16, tag="y", name=key_y)
                            y_sb = state[key_y]
                            nc.vector.tensor_copy(
                                out=y_sb[:, 512 * half:512 * half + w], in_=ps[:, 0:w])
                            if half == 1:
                                nc.sync.dma_start(
                                    y_d[128 * st:128 * (st + 1), :], y_sb[:])
                    return chunk

                out = []
                for half in halves:
                    out.append(mk(half, (0, 1), False))
                    out.append(mk(half, (2, 3), False))
                    out.append(mk(half, (4, 5), True))
                return out

            fillers = []

            def pop_fillers(n):
                for _ in range(n):
                    if fillers:
                        fillers.pop(0)()

            def drain_fillers():
                while fillers:
                    fillers.pop(0)()

            # ---- attention panel (hp, qw) accumulating into av[hh] ----
            def attention(hp, qw, av, rate):
                colr = 256 * (qw % 2)
                nkc = 2 * qw + 2
                pts = {}

                def emit_av(kc):
                    # av[hh] is a single accumulation group per PSUM bank
                    # spanning both q panels of the qc pair (one start, one
                    # stop) — hardware PSUM start/stop state is bank-granular.
                    for hh in (0, 1):
                        h = 2 * hp + hh
                        nc.tensor.matmul(
                            av[hh][:, colr:colr + 256],
                            vp[:, kc, 65 * h:65 * (h + 1)],
                            pts[kc][:, hh, :],
                            start=(qw % 2 == 0 and kc == 0),
                            stop=(qw % 2 == 1 and kc == nkc - 1),
                            skip_group_check=True)

                for kc in range(nkc):
                    # [128, 2, 512] spans 2 PSUM banks: one bank (and one
                    # accumulation group) per hh; only cols 0:256 are used.
                    ps = pss.tile([128, 2, 512], F32, tag="ps")
                    for hh in (0, 1):
                        rows = slice(64 * hh, 64 * (hh + 1))
                        nc.tensor.matmul(
                            ps[:, hh, 0:256],
                            kT[rows, hp, 128 * kc:128 * (kc + 1)],
                            qT[rows, hp, 256 * qw:256 * (qw + 1)],
                            start=True, stop=True,
                            tile_position=(64 * hh, 0))
                    pt = ptp.tile([128, 2, 256], BF16, tag="pt")
                    nc.scalar.activation(pt[:], ps[:, :, 0:256], AF.Exp)
                    if kc >= 2 * qw:
                        j = kc - 2 * qw
                        mh = mhp.tile([128, 2, 256], BF16, tag="mh")
                        nc.gpsimd.tensor_tensor(mh[:], pt[:], msk[:, j], MULT)
                        pts[kc] = mh
                    else:
                        pts[kc] = pt
                    if kc > 0:
                        emit_av(kc - 1)
                    pop_fillers(rate)
                emit_av(nkc - 1)

            deferred = []

            def flush_deferred():
                while deferred:
                    deferred.pop(0)()

            def div_unit(hp, qc, av):
                # emit reciprocal + broadcast DMAs now; defer the final
                # multiply (DVE) so it never heads the DVE FIFO while its
                # broadcast DMA is still in flight.
                for hh in (0, 1):
                    h = 2 * hp + hh
                    den = smallp.tile([1, 512], F32, tag="den", name=f"dn{h}_{qc}")
                    nc.vector.tensor_copy(out=den[:], in_=av[hh][64:65, :])
                    rc = smallp.tile([1, 512], F32, tag="rc", name=f"rc{h}_{qc}")
                    nc.vector.reciprocal_approx_fast(out=rc[:], in_=den[:])
                    nc.sync.dma_start(recip_d[h, qc, :], rc[:])
                    rb = rbp.tile([64, 512], F32, tag="rb", name=f"rb{h}_{qc}")
                    sl = recip_d[h, qc, :]
                    bc = bass.AP(tensor=sl.tensor, offset=sl.offset,
                                 ap=[[0, 64]] + list(sl.ap))
                    nc.sync.dma_start(rb[:], bc)

                    def tt(hh=hh, rb=rb):
                        rows = slice(64 * hh, 64 * (hh + 1))
                        nc.vector.tensor_tensor(
                            outT[rows, hp, 512 * qc:512 * (qc + 1)],
                            av[hh][0:64, :], rb[:], MULT)
                    deferred.append(tt)

            # ---- PRE-min: just K/Q proj (nt=0, sc=0) and V st0,1 ----
            for ch in proj_unit_chunks(wk_sb, kT, 0, 0, "pss"):
                ch()
            for ch in proj_unit_chunks(wq_sb, qT, 0, 0, "pss"):
                ch()
            for st in (0, 1):
                for ch in v_unit_chunks(st, "pss"):
                    ch()

            # F1 fillers (rate=2 over qc0): remaining sc0 K/Q proj ahead of
            # their head pair, V st4..7, then K/Q (nt=0, sc=1) for qc1.
            for nt in (1, 2, 3, 4, 5):
                fillers.extend(proj_unit_chunks(wk_sb, kT, nt, 0, "pproj"))
                fillers.extend(proj_unit_chunks(wq_sb, qT, nt, 0, "pproj"))
            for a, b in ((4, 5), (6, 7)):
                fillers.extend(v_unit_chunks(a, "pproj", halves=(0,)))
                fillers.extend(v_unit_chunks(b, "pproj", halves=(0,)))
                fillers.extend(v_unit_chunks(a, "pproj", halves=(1,)))
                fillers.extend(v_unit_chunks(b, "pproj", halves=(1,)))
            fillers.extend(proj_unit_chunks(wk_sb, kT, 0, 1, "pproj"))
            fillers.extend(proj_unit_chunks(wq_sb, qT, 0, 1, "pproj"))

            # ---- attention qc0 (qw 0,1) ----
            for hp in range(NT):
                flush_deferred()
                av = {hh: pav.tile([65, 512], F32, tag="av", name=f"av0_{hp}_{hh}")
                      for hh in (0, 1)}
                if hp == 0:
                    attention(hp, 0, av, rate=0)
                    # V st2,3 needed from qw1 on; emit directly (pss free now)
                    for st in (2, 3):
                        for ch in v_unit_chunks(st, "pss"):
                            ch()
                    attention(hp, 1, av, rate=2)
                else:
                    attention(hp, 0, av, rate=2)
                    attention(hp, 1, av, rate=2)
                div_unit(hp, 0, av)

            # F2 fillers: remaining K/Q proj (sc=1) staggered ahead of their
            # consuming head pair, and output projection for s rows 0:512.
            f2 = []
            f2.append(proj_unit_chunks(wk_sb, kT, 1, 1, "pproj")
                      + proj_unit_chunks(wq_sb, qT, 1, 1, "pproj"))
            f2.append(proj_unit_chunks(wk_sb, kT, 2, 1, "pproj")
                      + proj_unit_chunks(wq_sb, qT, 2, 1, "pproj"))
            f2.append(ste_unit_chunks(0, "pproj", halves=(0,))
                      + ste_unit_chunks(1, "pproj", halves=(0,))
                      + ste_unit_chunks(0, "pproj", halves=(1,))
                      + ste_unit_chunks(1, "pproj", halves=(1,)))
            f2.append(proj_unit_chunks(wk_sb, kT, 3, 1, "pproj")
                      + proj_unit_chunks(wq_sb, qT, 3, 1, "pproj"))
            f2.append(ste_unit_chunks(2, "pproj", halves=(0,))
                      + ste_unit_chunks(3, "pproj", halves=(0,))
                      + ste_unit_chunks(2, "pproj", halves=(1,))
                      + ste_unit_chunks(3, "pproj", halves=(1,)))
            f2.append(proj_unit_chunks(wk_sb, kT, 4, 1, "pproj")
                      + proj_unit_chunks(wq_sb, qT, 4, 1, "pproj"))
            f2.append(proj_unit_chunks(wk_sb, kT, 5, 1, "pproj")
                      + proj_unit_chunks(wq_sb, qT, 5, 1, "pproj"))
            for grp in f2:
                fillers.extend(grp)

            # ---- attention qc1 (qw 2,3) ----
            for hp in range(NT):
                flush_deferred()
                av = {hh: pav.tile([65, 512], F32, tag="av", name=f"av1_{hp}_{hh}")
                      for hh in (0, 1)}
                attention(hp, 2, av, rate=1)
                attention(hp, 3, av, rate=1)
                div_unit(hp, 1, av)

            drain_fillers()
            flush_deferred()

            # ---- tail: output projection for s rows 512:1024 ----
            for st in range(4, ST):
                for ch in ste_unit_chunks(st, "pss"):
                    ch()

    nc.compile()
    return nc


def _get_compiled():
    global _compiled
    if _compiled is None:
        _compiled = _build_nc()
    return _compiled


def _prep_in_maps(x, W_attn, W_proj):
    import ml_dtypes

    BF = ml_dtypes.bfloat16
    x = np.asarray(x, dtype=np.float32)
    W_attn = np.asarray(W_attn, dtype=np.float32)
    W_proj = np.asarray(W_proj, dtype=np.float32)

    def wlayout(w):
        return np.ascontiguousarray(
            w.reshape(NT, 128, D).transpose(1, 0, 2)).astype(BF)

    wq = wlayout(np.ascontiguousarray(W_attn[:, 0:D]) * np.float32(0.125))
    wk = wlayout(np.ascontiguousarray(W_attn[:, D:2 * D]))
    wv = wlayout(np.ascontiguousarray(W_attn[:, 2 * D:3 * D]))
    wp = wlayout(W_proj)
    masks = _build_mask()

    xT = np.transpose(x, (0, 2, 1)).reshape(B, NT, 128, S).transpose(0, 2, 1, 3)
    xT = np.ascontiguousarray(xT).astype(BF)

    return [
        {"xT": xT[b], "wq": wq, "wk": wk, "wv": wv, "wp": wp, "masks": masks}
        for b in range(B)
    ]


def kernel(x, W_attn, W_proj):
    from concourse.bass_utils import run_bass_kernel_spmd

    nc = _get_compiled()
    in_maps = _prep_in_maps(x, W_attn, W_proj)
    res = run_bass_kernel_spmd(nc, in_maps, list(range(NC_)))
    y = np.stack([res.results[b]["y"] for b in range(B)], axis=0)
    return y.astype(np.float32)


# revision 15
# speedup vs baseline: 1.0359x; 1.0359x over previous
"""Multi-head causal attention (B=8, S=1024, D=768, H=12) on 8 trn2 NeuronCores.

Strategy: data-parallel over batch (one batch element per core, no collectives).

v2 design (all-bf16 operands, fp32 PSUM accumulation):
  - bf16 x/W halve input DMA (6MB vs 16MB) and enable PE fast-weight-load.
  - causal blocking at (128 kpos x 256 q) granularity: 0.625 of S^2 computed
    (vs 0.75 at 512-q blocks), scores as S^T[k,q] = K @ Q^T per head, two
    heads packed into the 128-row PE array via tile_position row tiling.
  - softmax: exp on ScalarE from PSUM [128, 2, 256] tiles; 1/8 scale folded
    into W_q host-side; 0/1 bf16 mask multiply (DVE) only on the two
    diagonal-crossing kpos blocks of each q panel.
  - A@V with ones column appended to V (denominator rides row 64 of the
    [65, 512] PSUM); division folded into the PSUM->SBUF copy (fast
    reciprocal + DMA partition-broadcast through a DRAM scratch).
  - phase schedule keeps the PE dense (HAM warm): PRE emits V-proj st0..3 +
    Q/K proj for q columns 0:512; attention runs q panels ascending
    (qc0: qw0,qw1 then qc1: qw2,qw3) with remaining V-proj/Q/K-proj/output-
    projection work woven between attention steps; y rows stream out as
    their q panel completes.
"""
import sys

if "/opt/trn_rl_repo" not in sys.path:
    sys.path.insert(0, "/opt/trn_rl_repo")

import numpy as np

B, S, D, H = 8, 1024, 768, 12
DH = 64
NC_ = 8
NT = D // 128     # 6
ST = S // 128     # 8
QW = S // 256     # 4 q panels
QC = S // 512     # 2

_compiled = None


def _build_mask():
    import ml_dtypes

    p = np.arange(128)[:, None, None, None]
    j = np.arange(2)[None, :, None, None]
    hh = np.arange(2)[None, None, :, None]
    c = np.arange(256)[None, None, None, :]
    m = (c >= 128 * j + p).astype(np.float32) + 0 * hh
    return np.broadcast_to(m, (128, 2, 2, 256)).astype(ml_dtypes.bfloat16)


def _build_nc():
    import concourse.bass as bass
    import concourse.mybir as mybir
    import concourse.tile as tile
    from concourse import bacc

    F32 = mybir.dt.float32
    BF16 = mybir.dt.bfloat16
    AF = mybir.ActivationFunctionType
    MULT = mybir.AluOpType.mult

    nc = bacc.Bacc("TRN2", target_bir_lowering=False, debug=False)

    xT_d = nc.dram_tensor("xT", [128, NT, S], BF16, kind="ExternalInput")
    wq_d = nc.dram_tensor("wq", [128, NT, D], BF16, kind="ExternalInput")
    wk_d = nc.dram_tensor("wk", [128, NT, D], BF16, kind="ExternalInput")
    wv_d = nc.dram_tensor("wv", [128, NT, D], BF16, kind="ExternalInput")
    wp_d = nc.dram_tensor("wp", [128, NT, D], BF16, kind="ExternalInput")
    mask_d = nc.dram_tensor("masks", [128, 2, 2, 256], BF16, kind="ExternalInput")
    y_d = nc.dram_tensor("y", [S, D], BF16, kind="ExternalOutput")
    recip_d = nc.dram_tensor("recip_scratch", [H, QC, 512], F32)

    with tile.TileContext(nc) as tc:
        with (
            tc.tile_pool(name="static", bufs=1) as static,
            tc.tile_pool(name="pt", bufs=5) as ptp,
            tc.tile_pool(name="mh", bufs=3) as mhp,
            tc.tile_pool(name="small", bufs=4) as smallp,
            tc.tile_pool(name="rb", bufs=3) as rbp,
            tc.tile_pool(name="y", bufs=2) as ypool,
            tc.tile_pool(name="pss", bufs=2, space="PSUM") as pss,
            tc.tile_pool(name="pav", bufs=3, space="PSUM") as pav,
            tc.tile_pool(name="pproj", bufs=1, space="PSUM") as pproj,
        ):
            # ---- persistent SBUF ----
            xT = static.tile([128, NT, S], BF16)
            qT = static.tile([128, NT, S], BF16)
            kT = static.tile([128, NT, S], BF16)
            outT = static.tile([128, NT, S], BF16)
            vp = static.tile([128, ST, H * (DH + 1)], BF16)
            msk = static.tile([128, 2, 2, 256], BF16)
            wq_sb = static.tile([128, NT, D], BF16)
            wk_sb = static.tile([128, NT, D], BF16)
            wv_sb = static.tile([128, NT, D], BF16)
            wp_sb = static.tile([128, NT, D], BF16)

            # input DMAs split across the two hwdge queues (sync + scalar)
            for dc in (0, 2, 4):
                nc.sync.dma_start(xT[:, dc, :], xT_d[:, dc, :])
            for dc in (1, 3, 5):
                nc.scalar.dma_start(xT[:, dc, :], xT_d[:, dc, :])
            nc.sync.dma_start(msk[:], mask_d[:])
            nc.scalar.dma_start(wk_sb[:], wk_d[:])
            nc.sync.dma_start(wq_sb[:], wq_d[:])
            nc.scalar.dma_start(wv_sb[:], wv_d[:])
            nc.sync.dma_start(wp_sb[:], wp_d[:])
            nc.vector.memset(vp[:], 1.0)

            # ---- work units, emitted as ~2-matmul chunks ----
            # pool="pss": [128, 2, 512] 2-bank tile, halves A=[:,0,:] (512)
            #   B=[:,1,0:256] — used in PRE/tail when scores are idle.
            # pool="pproj": single-bank [128, 512] tiles — used for fillers
            #   woven into attention (A and B are separate staggered units).
            uid = [0]

            def v_unit_chunks(st, pool, halves=(0, 1)):
                state = {}

                def mk(half, dcs, fin):
                    def chunk():
                        key = f"v{st}_{half}"
                        if key not in state:
                            if pool == "pss":
                                if "t" not in state:
                                    state["t"] = pss.tile([128, 2, 512], F32, tag="ps", name=f"vps{st}")
                                state[key] = state["t"][:, half, 0:512 if half == 0 else 256]
                            else:
                                t = pproj.tile([128, 512], F32, tag="proj", name=key)
                                state[key] = t[:, 0:512 if half == 0 else 256]
                        ps = state[key]
                        w = 512 if half == 0 else 256
                        for dc in dcs:
                            nc.tensor.matmul(
                                ps[:, 0:w], xT[:, dc, 128 * st:128 * (st + 1)],
                                wv_sb[:, dc, 512 * half:512 * half + w],
                                start=(dc == 0), stop=(dc == NT - 1))
                        if fin:
                            dst = vp[:, st, :].rearrange("p (h e) -> p h e", e=DH + 1)
                            if half == 0:
                                nc.vector.tensor_copy(
                                    out=dst[:, 0:8, 0:DH],
                                    in_=ps[:, 0:512].rearrange("p (h d) -> p h d", d=DH))
                            else:
                                nc.vector.tensor_copy(
                                    out=dst[:, 8:12, 0:DH],
                                    in_=ps[:, 0:256].rearrange("p (h d) -> p h d", d=DH))
                    return chunk

                out = []
                for half in halves:
                    out.append(mk(half, (0, 1), False))
                    out.append(mk(half, (2, 3), False))
                    out.append(mk(half, (4, 5), True))
                return out

            def proj_unit_chunks(w_sb, dst, nt, sc, pool):
                state = {}

                def mk(dcs, fin):
                    def chunk():
                        if "ps" not in state:
                            uid[0] += 1
                            if pool == "pss":
                                t = pss.tile([128, 2, 512], F32, tag="ps", name=f"qk{uid[0]}")
                                state["ps"] = t[:, 0, :]
                            else:
                                state["ps"] = pproj.tile([128, 512], F32, tag="proj", name=f"qk{uid[0]}")
                        ps = state["ps"]
                        for dc in dcs:
                            nc.tensor.matmul(
                                ps[:],
                                w_sb[:, dc, 128 * nt:128 * (nt + 1)],
                                xT[:, dc, 512 * sc:512 * (sc + 1)],
                                start=(dc == 0), stop=(dc == NT - 1))
                        if fin:
                            nc.vector.tensor_copy(
                                out=dst[:, nt, 512 * sc:512 * (sc + 1)], in_=ps[:])
                    return chunk

                return [mk((0, 1), False), mk((2, 3), False), mk((4, 5), True)]

            def ste_unit_chunks(st, pool, halves=(0, 1)):
                state = {}

                def mk(half, dcs, fin):
                    def chunk():
                        key = f"e{st}_{half}"
                        if key not in state:
                            if pool == "pss":
                                if "t" not in state:
                                    state["t"] = pss.tile([128, 2, 512], F32, tag="ps", name=f"eps{st}")
                                state[key] = state["t"][:, half, 0:512 if half == 0 else 256]
                            else:
                                t = pproj.tile([128, 512], F32, tag="proj", name=key)
                                state[key] = t[:, 0:512 if half == 0 else 256]
                        ps = state[key]
                        w = 512 if half == 0 else 256
                        for dc in dcs:
                            nc.tensor.matmul(
                                ps[:, 0:w], outT[:, dc, 128 * st:128 * (st + 1)],
                                wp_sb[:, dc, 512 * half:512 * half + w],
                                start=(dc == 0), stop=(dc == NT - 1))
                        if fin:
                            key_y = f"y{st}"
                            if key_y not in state:
                                state[key_y] = ypool.tile([128, D], BF16, tag="y", name=key_y)
                            y_sb = state[key_y]
                            nc.vector.tensor_copy(
                                out=y_sb[:, 512 * half:512 * half + w], in_=ps[:, 0:w])
                            if half == 1:
                                nc.sync.dma_start(
                                    y_d[128 * st:128 * (st + 1), :], y_sb[:])
                    return chunk

                out = []
                for half in halves:
                    out.append(mk(half, (0, 1), False))
                    out.append(mk(half, (2, 3), False))
                    out.append(mk(half, (4, 5), True))
                return out

            fillers = []

            def pop_fillers(n):
                for _ in range(n):
                    if fillers:
                        fillers.pop(0)()

            def drain_fillers():
                while fillers:
                    fillers.pop(0)()

            # ---- attention panel (hp, qw) accumulating into av[hh] ----
            # kc runs DESCENDING so the diagonal (masked) blocks are first:
            # their exp->mask latency hides behind the later blocks' scores,
            # and the panel never ends waiting on a mask. A@V emission lags
            # the scores by 2 blocks.
            def attention(hp, qw, av, rate):
                colr = 256 * (qw % 2)
                nkc = 2 * qw + 2
                pts = {}

                def emit_av(kc):
                    for hh in (0, 1):
                        h = 2 * hp + hh
                        nc.tensor.matmul(
                            av[hh][:, colr:colr + 256],
                            vp[:, kc, 65 * h:65 * (h + 1)],
                            pts[kc][:, hh, :],
                            start=(qw % 2 == 0 and kc == nkc - 1),
                            stop=(qw % 2 == 1 and kc == 0),
                            skip_group_check=True)

                pending = []
                for kc in range(nkc - 1, -1, -1):
                    # [128, 2, 512] spans 2 PSUM banks: one bank (and one
                    # accumulation group) per hh; only cols 0:256 are used.
                    ps = pss.tile([128, 2, 512], F32, tag="ps")
                    for hh in (0, 1):
                        rows = slice(64 * hh, 64 * (hh + 1))
                        nc.tensor.matmul(
                            ps[:, hh, 0:256],
                            kT[rows, hp, 128 * kc:128 * (kc + 1)],
                            qT[rows, hp, 256 * qw:256 * (qw + 1)],
                            start=True, stop=True,
                            tile_position=(64 * hh, 0))
                    pt = ptp.tile([128, 2, 256], BF16, tag="pt")
                    nc.scalar.activation(pt[:], ps[:, :, 0:256], AF.Exp)
                    if kc >= 2 * qw:
                        j = kc - 2 * qw
                        mh = mhp.tile([128, 2, 256], BF16, tag="mh")
                        for hh in (0, 1):
                            nc.gpsimd.tensor_tensor(
                                mh[:, hh, :], pt[:, hh, :], msk[:, j, hh, :], MULT)
                        pts[kc] = mh
                    else:
                        pts[kc] = pt
                    pending.append(kc)
                    if len(pending) > 2:
                        emit_av(pending.pop(0))
                    pop_fillers(rate)
                while pending:
                    emit_av(pending.pop(0))
                    pop_fillers(1)

            deferred = []

            def flush_deferred():
                while deferred:
                    deferred.pop(0)()

            def div_unit(hp, qc, av):
                # emit reciprocal + broadcast DMAs now; defer the final
                # multiply (DVE) so it never heads the DVE FIFO while its
                # broadcast DMA is still in flight.
                for hh in (0, 1):
                    h = 2 * hp + hh
                    den = smallp.tile([1, 512], F32, tag="den", name=f"dn{h}_{qc}")
                    nc.vector.tensor_copy(out=den[:], in_=av[hh][64:65, :])
                    rc = smallp.tile([1, 512], F32, tag="rc", name=f"rc{h}_{qc}")
                    nc.vector.reciprocal_approx_fast(out=rc[:], in_=den[:])
                    nc.sync.dma_start(recip_d[h, qc, :], rc[:])
                    rb = rbp.tile([64, 512], F32, tag="rb", name=f"rb{h}_{qc}")
                    sl = recip_d[h, qc, :]
                    bc = bass.AP(tensor=sl.tensor, offset=sl.offset,
                                 ap=[[0, 64]] + list(sl.ap))
                    nc.sync.dma_start(rb[:], bc)

                    def tt(hh=hh, rb=rb):
                        rows = slice(64 * hh, 64 * (hh + 1))
                        nc.vector.tensor_tensor(
                            outT[rows, hp, 512 * qc:512 * (qc + 1)],
                            av[hh][0:64, :], rb[:], MULT)
                    deferred.append(tt)

            # ---- PRE-min: just K/Q proj (nt=0, sc=0) and V st0,1 ----
            for ch in proj_unit_chunks(wk_sb, kT, 0, 0, "pss"):
                ch()
            for ch in proj_unit_chunks(wq_sb, qT, 0, 0, "pss"):
                ch()
            for st in (0, 1):
                for ch in v_unit_chunks(st, "pss"):
                    ch()

            # F1 fillers (rate=2 over qc0): remaining sc0 K/Q proj ahead of
            # their head pair, V st4..7, then K/Q (nt=0, sc=1) for qc1.
            for nt in (1, 2, 3, 4, 5):
                fillers.extend(proj_unit_chunks(wk_sb, kT, nt, 0, "pproj"))
                fillers.extend(proj_unit_chunks(wq_sb, qT, nt, 0, "pproj"))
            for a, b in ((4, 5), (6, 7)):
                fillers.extend(v_unit_chunks(a, "pproj", halves=(0,)))
                fillers.extend(v_unit_chunks(b, "pproj", halves=(0,)))
                fillers.extend(v_unit_chunks(a, "pproj", halves=(1,)))
                fillers.extend(v_unit_chunks(b, "pproj", halves=(1,)))
            fillers.extend(proj_unit_chunks(wk_sb, kT, 0, 1, "pproj"))
            fillers.extend(proj_unit_chunks(wq_sb, qT, 0, 1, "pproj"))

            # ---- attention qc0 (qw 0,1) ----
            for hp in range(NT):
                flush_deferred()
                av = {hh: pav.tile([65, 512], F32, tag="av", name=f"av0_{hp}_{hh}")
                      for hh in (0, 1)}
                if hp == 0:
                    attention(hp, 0, av, rate=0)
                    # V st2,3 needed from qw1 on; emit directly (pss free now)
                    for st in (2, 3):
                        for ch in v_unit_chunks(st, "pss"):
                            ch()
                    attention(hp, 1, av, rate=2)
                else:
                    attention(hp, 0, av, rate=2)
                    attention(hp, 1, av, rate=2)
                div_unit(hp, 0, av)

            # F2 fillers: remaining K/Q proj (sc=1) staggered ahead of their
            # consuming head pair, and output projection for s rows 0:512.
            f2 = []
            f2.append(proj_unit_chunks(wk_sb, kT, 1, 1, "pproj")
                      + proj_unit_chunks(wq_sb, qT, 1, 1, "pproj"))
            f2.append(proj_unit_chunks(wk_sb, kT, 2, 1, "pproj")
                      + proj_unit_chunks(wq_sb, qT, 2, 1, "pproj"))
            f2.append(ste_unit_chunks(0, "pproj", halves=(0,))
                      + ste_unit_chunks(1, "pproj", halves=(0,))
                      + ste_unit_chunks(0, "pproj", halves=(1,))
                      + ste_unit_chunks(1, "pproj", halves=(1,)))
            f2.append(proj_unit_chunks(wk_sb, kT, 3, 1, "pproj")
                      + proj_unit_chunks(wq_sb, qT, 3, 1, "pproj"))
            f2.append(ste_unit_chunks(2, "pproj", halves=(0,))
                      + ste_unit_chunks(3, "pproj", halves=(0,))
                      + ste_unit_chunks(2, "pproj", halves=(1,))
                      + ste_unit_chunks(3, "pproj", halves=(1,)))
            f2.append(proj_unit_chunks(wk_sb, kT, 4, 1, "pproj")
                      + proj_unit_chunks(wq_sb, qT, 4, 1, "pproj"))
            f2.append(proj_unit_chunks(wk_sb, kT, 5, 1, "pproj")
                      + proj_unit_chunks(wq_sb, qT, 5, 1, "pproj"))
            for grp in f2:
                fillers.extend(grp)

            # ---- attention qc1 (qw 2,3) ----
            for hp in range(NT):
                flush_deferred()
                av = {hh: pav.tile([65, 512], F32, tag="av", name=f"av1_{hp}_{hh}")
                      for hh in (0, 1)}
                attention(hp, 2, av, rate=1)
                attention(hp, 3, av, rate=1)
                div_unit(hp, 1, av)

            drain_fillers()
            flush_deferred()

            # ---- tail: output projection for s rows 512:1024 ----
            for st in range(4, ST):
                for ch in ste_unit_chunks(st, "pss"):
                    ch()

    nc.compile()
    return nc


def _get_compiled():
    global _compiled
    if _compiled is None:
        _compiled = _build_nc()
    return _compiled


def _prep_in_maps(x, W_attn, W_proj):
    import ml_dtypes

    BF = ml_dtypes.bfloat16
    x = np.asarray(x, dtype=np.float32)
    W_attn = np.asarray(W_attn, dtype=np.float32)
    W_proj = np.asarray(W_proj, dtype=np.float32)

    def wlayout(w):
        return np.ascontiguousarray(
            w.reshape(NT, 128, D).transpose(1, 0, 2)).astype(BF)

    wq = wlayout(np.ascontiguousarray(W_attn[:, 0:D]) * np.float32(0.125))
    wk = wlayout(np.ascontiguousarray(W_attn[:, D:2 * D]))
    wv = wlayout(np.ascontiguousarray(W_attn[:, 2 * D:3 * D]))
    wp = wlayout(W_proj)
    masks = _build_mask()

    xT = np.transpose(x, (0, 2, 1)).reshape(B, NT, 128, S).transpose(0, 2, 1, 3)
    xT = np.ascontiguousarray(xT).astype(BF)

    return [
        {"xT": xT[b], "wq": wq, "wk": wk, "wv": wv, "wp": wp, "masks": masks}
        for b in range(B)
    ]


def kernel(x, W_attn, W_proj):
    from concourse.bass_utils import run_bass_kernel_spmd

    nc = _get_compiled()
    in_maps = _prep_in_maps(x, W_attn, W_proj)
    res = run_bass_kernel_spmd(nc, in_maps, list(range(NC_)))
    y = np.stack([res.results[b]["y"] for b in range(B)], axis=0)
    return y.astype(np.float32)


# revision 17
# speedup vs baseline: 1.0931x; 1.0552x over previous
"""Multi-head causal attention (B=8, S=1024, D=768, H=12) on 8 trn2 NeuronCores.

Strategy: data-parallel over batch (one batch element per core, no collectives).

v2 design (all-bf16 operands, fp32 PSUM accumulation):
  - bf16 x/W halve input DMA (6MB vs 16MB) and enable PE fast-weight-load.
  - causal blocking at (128 kpos x 256 q) granularity: 0.625 of S^2 computed
    (vs 0.75 at 512-q blocks), scores as S^T[k,q] = K @ Q^T per head, two
    heads packed into the 128-row PE array via tile_position row tiling.
  - softmax: exp on ScalarE from PSUM [128, 2, 256] tiles; 1/8 scale folded
    into W_q host-side; 0/1 bf16 mask multiply (DVE) only on the two
    diagonal-crossing kpos blocks of each q panel.
  - A@V with ones column appended to V (denominator rides row 64 of the
    [65, 512] PSUM); division folded into the PSUM->SBUF copy (fast
    reciprocal + DMA partition-broadcast through a DRAM scratch).
  - phase schedule keeps the PE dense (HAM warm): PRE emits V-proj st0..3 +
    Q/K proj for q columns 0:512; attention runs q panels ascending
    (qc0: qw0,qw1 then qc1: qw2,qw3) with remaining V-proj/Q/K-proj/output-
    projection work woven between attention steps; y rows stream out as
    their q panel completes.
"""
import sys

if "/opt/trn_rl_repo" not in sys.path:
    sys.path.insert(0, "/opt/trn_rl_repo")

import numpy as np

B, S, D, H = 8, 1024, 768, 12
DH = 64
NC_ = 8
NT = D // 128     # 6
ST = S // 128     # 8
QW = S // 256     # 4 q panels
QC = S // 512     # 2

_compiled = None


def _build_mask():
    import ml_dtypes

    p = np.arange(128)[:, None, None, None]
    j = np.arange(2)[None, :, None, None]
    hh = np.arange(2)[None, None, :, None]
    c = np.arange(256)[None, None, None, :]
    m = (c >= 128 * j + p).astype(np.float32) + 0 * hh
    return np.broadcast_to(m, (128, 2, 2, 256)).astype(ml_dtypes.bfloat16)


def _build_nc():
    import concourse.bass as bass
    import concourse.mybir as mybir
    import concourse.tile as tile
    from concourse import bacc

    F32 = mybir.dt.float32
    BF16 = mybir.dt.bfloat16
    AF = mybir.ActivationFunctionType
    MULT = mybir.AluOpType.mult

    nc = bacc.Bacc("TRN2", target_bir_lowering=False, debug=False)

    xT_d = nc.dram_tensor("xT", [128, NT, S], BF16, kind="ExternalInput")
    wq_d = nc.dram_tensor("wq", [128, NT, D], BF16, kind="ExternalInput")
    wk_d = nc.dram_tensor("wk", [128, NT, D], BF16, kind="ExternalInput")
    wv_d = nc.dram_tensor("wv", [128, NT, D], BF16, kind="ExternalInput")
    wp_d = nc.dram_tensor("wp", [128, NT, D], BF16, kind="ExternalInput")
    mask_d = nc.dram_tensor("masks", [128, 2, 2, 256], BF16, kind="ExternalInput")
    y_d = nc.dram_tensor("y", [S, D], BF16, kind="ExternalOutput")
    recip_d = nc.dram_tensor("recip_scratch", [H, QC, 512], F32)

    with tile.TileContext(nc) as tc:
        with (
            tc.tile_pool(name="static", bufs=1) as static,
            tc.tile_pool(name="pt", bufs=5) as ptp,
            tc.tile_pool(name="mh", bufs=3) as mhp,
            tc.tile_pool(name="small", bufs=4) as smallp,
            tc.tile_pool(name="rb", bufs=3) as rbp,
            tc.tile_pool(name="y", bufs=2) as ypool,
            tc.tile_pool(name="pss", bufs=2, space="PSUM") as pss,
            tc.tile_pool(name="pav", bufs=3, space="PSUM") as pav,
            tc.tile_pool(name="pproj", bufs=1, space="PSUM") as pproj,
        ):
            # ---- persistent SBUF ----
            xT = static.tile([128, NT, S], BF16)
            qT = static.tile([128, NT, S], BF16)
            kT = static.tile([128, NT, S], BF16)
            outT = static.tile([128, NT, S], BF16)
            vp = static.tile([128, ST, H * (DH + 1)], BF16)
            msk = static.tile([128, 2, 2, 256], BF16)
            wq_sb = static.tile([128, NT, D], BF16)
            wk_sb = static.tile([128, NT, D], BF16)
            wv_sb = static.tile([128, NT, D], BF16)
            wp_sb = static.tile([128, NT, D], BF16)

            # input DMAs split across the two hwdge queues; one DMA per
            # tensor so descriptors stay large (12KB/partition for x).
            nc.sync.dma_start(xT[:], xT_d[:])
            nc.scalar.dma_start(wk_sb[:], wk_d[:])
            nc.scalar.dma_start(wq_sb[:], wq_d[:])
            nc.sync.dma_start(wv_sb[:], wv_d[:])
            nc.scalar.dma_start(msk[:], mask_d[:])
            nc.scalar.dma_start(wp_sb[:], wp_d[:])
            nc.vector.memset(vp[:], 1.0)

            # ---- work units, emitted as ~2-matmul chunks ----
            # pool="pss": [128, 2, 512] 2-bank tile, halves A=[:,0,:] (512)
            #   B=[:,1,0:256] — used in PRE/tail when scores are idle.
            # pool="pproj": single-bank [128, 512] tiles — used for fillers
            #   woven into attention (A and B are separate staggered units).
            uid = [0]

            def v_unit_chunks(st, pool, halves=(0, 1)):
                state = {}

                def mk(half, dcs, fin):
                    def chunk():
                        key = f"v{st}_{half}"
                        if key not in state:
                            if pool == "pss":
                                if "t" not in state:
                                    state["t"] = pss.tile([128, 2, 512], F32, tag="ps", name=f"vps{st}")
                                state[key] = state["t"][:, half, 0:512 if half == 0 else 256]
                            else:
                                t = pproj.tile([128, 512], F32, tag="proj", name=key)
                                state[key] = t[:, 0:512 if half == 0 else 256]
                        ps = state[key]
                        w = 512 if half == 0 else 256
                        for dc in dcs:
                            nc.tensor.matmul(
                                ps[:, 0:w], xT[:, dc, 128 * st:128 * (st + 1)],
                                wv_sb[:, dc, 512 * half:512 * half + w],
                                start=(dc == 0), stop=(dc == NT - 1))
                        if fin:
                            dst = vp[:, st, :].rearrange("p (h e) -> p h e", e=DH + 1)
                            if half == 0:
                                nc.vector.tensor_copy(
                                    out=dst[:, 0:8, 0:DH],
                                    in_=ps[:, 0:512].rearrange("p (h d) -> p h d", d=DH))
                            else:
                                nc.vector.tensor_copy(
                                    out=dst[:, 8:12, 0:DH],
                                    in_=ps[:, 0:256].rearrange("p (h d) -> p h d", d=DH))
                    return chunk

                out = []
                for half in halves:
                    out.append(mk(half, (0, 1), False))
                    out.append(mk(half, (2, 3), False))
                    out.append(mk(half, (4, 5), True))
                return out

            def proj_unit_chunks(w_sb, dst, nt, sc, pool):
                state = {}

                def mk(dcs, fin):
                    def chunk():
                        if "ps" not in state:
                            uid[0] += 1
                            if pool == "pss":
                                t = pss.tile([128, 2, 512], F32, tag="ps", name=f"qk{uid[0]}")
                                state["ps"] = t[:, 0, :]
                            else:
                                state["ps"] = pproj.tile([128, 512], F32, tag="proj", name=f"qk{uid[0]}")
                        ps = state["ps"]
                        for dc in dcs:
                            nc.tensor.matmul(
                                ps[:],
                                w_sb[:, dc, 128 * nt:128 * (nt + 1)],
                                xT[:, dc, 512 * sc:512 * (sc + 1)],
                                start=(dc == 0), stop=(dc == NT - 1))
                        if fin:
                            nc.vector.tensor_copy(
                                out=dst[:, nt, 512 * sc:512 * (sc + 1)], in_=ps[:])
                    return chunk

                return [mk((0, 1), False), mk((2, 3), False), mk((4, 5), True)]

            def ste_unit_chunks(st, pool, halves=(0, 1)):
                state = {}

                def mk(half, dcs, fin):
                    def chunk():
                        key = f"e{st}_{half}"
                        if key not in state:
                            if pool == "pss":
                                if "t" not in state:
                                    state["t"] = pss.tile([128, 2, 512], F32, tag="ps", name=f"eps{st}")
                                state[key] = state["t"][:, half, 0:512 if half == 0 else 256]
                            else:
                                t = pproj.tile([128, 512], F32, tag="proj", name=key)
                                state[key] = t[:, 0:512 if half == 0 else 256]
                        ps = state[key]
                        w = 512 if half == 0 else 256
                        for dc in dcs:
                            nc.tensor.matmul(
                                ps[:, 0:w], outT[:, dc, 128 * st:128 * (st + 1)],
                                wp_sb[:, dc, 512 * half:512 * half + w],
                                start=(dc == 0), stop=(dc == NT - 1))
                        if fin:
                            key_y = f"y{st}"
                            if key_y not in state:
                                state[key_y] = ypool.tile([128, D], BF16, tag="y", name=key_y)
                            y_sb = state[key_y]
                            nc.vector.tensor_copy(
                                out=y_sb[:, 512 * half:512 * half + w], in_=ps[:, 0:w])
                            if half == 1:
                                nc.sync.dma_start(
                                    y_d[128 * st:128 * (st + 1), :], y_sb[:])
                    return chunk

                out = []
                for half in halves:
                    out.append(mk(half, (0, 1), False))
                    out.append(mk(half, (2, 3), False))
                    out.append(mk(half, (4, 5), True))
                return out

            fillers = []

            def pop_fillers(n):
                for _ in range(n):
                    if fillers:
                        fillers.pop(0)()

            def drain_fillers():
                while fillers:
                    fillers.pop(0)()

            # ---- attention panel (hp, qw) accumulating into av[hh] ----
            # kc runs DESCENDING so the diagonal (masked) blocks are first:
            # their exp->mask latency hides behind the later blocks' scores,
            # and the panel never ends waiting on a mask. A@V emission lags
            # the scores by 2 blocks.
            def attention(hp, qw, av, rate):
                colr = 256 * (qw % 2)
                nkc = 2 * qw + 2
                pts = {}

                def emit_av(kc):
                    for hh in (0, 1):
                        h = 2 * hp + hh
                        nc.tensor.matmul(
                            av[hh][:, colr:colr + 256],
                            vp[:, kc, 65 * h:65 * (h + 1)],
                            pts[kc][:, hh, :],
                            start=(qw % 2 == 0 and kc == nkc - 1),
                            stop=(qw % 2 == 1 and kc == 0),
                            skip_group_check=True)

                pending = []
                for idx, kc in enumerate(range(nkc - 1, -1, -1)):
                    # [128, 2, 512] spans 2 PSUM banks: one bank (and one
                    # accumulation group) per hh; only cols 0:256 are used.
                    ps = pss.tile([128, 2, 512], F32, tag="ps")
                    for hh in (0, 1):
                        rows = slice(64 * hh, 64 * (hh + 1))
                        nc.tensor.matmul(
                            ps[:, hh, 0:256],
                            kT[rows, hp, 128 * kc:128 * (kc + 1)],
                            qT[rows, hp, 256 * qw:256 * (qw + 1)],
                            start=True, stop=True,
                            tile_position=(64 * hh, 0))
                    pt = ptp.tile([128, 2, 256], BF16, tag="pt")
                    nc.scalar.activation(pt[:], ps[:, :, 0:256], AF.Exp)
                    if kc >= 2 * qw:
                        j = kc - 2 * qw
                        mh = mhp.tile([128, 2, 256], BF16, tag="mh")
                        for hh in (0, 1):
                            nc.gpsimd.tensor_tensor(
                                mh[:, hh, :], pt[:, hh, :], msk[:, j, hh, :], MULT)
                        pts[kc] = mh
                    else:
                        pts[kc] = pt
                    pending.append(kc)
                    if len(pending) > 2:
                        emit_av(pending.pop(0))
                    if idx < 2:
                        pop_fillers(rate + 1)
                    elif idx < nkc - 2:
                        pop_fillers(rate)
                while pending:
                    emit_av(pending.pop(0))
                    pop_fillers(1)

            deferred = []

            def flush_deferred():
                while deferred:
                    deferred.pop(0)()

            def div_unit(hp, qc, av):
                # emit reciprocal + broadcast DMAs now; defer the final
                # multiply (DVE) so it never heads the DVE FIFO while its
                # broadcast DMA is still in flight.
                for hh in (0, 1):
                    h = 2 * hp + hh
                    den = smallp.tile([1, 512], F32, tag="den", name=f"dn{h}_{qc}")
                    nc.vector.tensor_copy(out=den[:], in_=av[hh][64:65, :])
                    rc = smallp.tile([1, 512], F32, tag="rc", name=f"rc{h}_{qc}")
                    nc.vector.reciprocal_approx_fast(out=rc[:], in_=den[:])
                    nc.sync.dma_start(recip_d[h, qc, :], rc[:])
                    rb = rbp.tile([64, 512], F32, tag="rb", name=f"rb{h}_{qc}")
                    sl = recip_d[h, qc, :]
                    bc = bass.AP(tensor=sl.tensor, offset=sl.offset,
                                 ap=[[0, 64]] + list(sl.ap))
                    nc.sync.dma_start(rb[:], bc)

                    def tt(hh=hh, rb=rb):
                        rows = slice(64 * hh, 64 * (hh + 1))
                        nc.vector.tensor_tensor(
                            outT[rows, hp, 512 * qc:512 * (qc + 1)],
                            av[hh][0:64, :], rb[:], MULT)
                    deferred.append(tt)

            # ---- PRE-min: just K/Q proj (nt=0, sc=0) and V st0,1 ----
            for ch in proj_unit_chunks(wk_sb, kT, 0, 0, "pss"):
                ch()
            for ch in proj_unit_chunks(wq_sb, qT, 0, 0, "pss"):
                ch()
            for st in (0, 1):
                for ch in v_unit_chunks(st, "pss"):
                    ch()

            # F1 fillers (rate=2 over qc0): remaining sc0 K/Q proj ahead of
            # their head pair, V st4..7, then K/Q (nt=0, sc=1) for qc1.
            for nt in (1, 2, 3, 4, 5):
                fillers.extend(proj_unit_chunks(wk_sb, kT, nt, 0, "pproj"))
                fillers.extend(proj_unit_chunks(wq_sb, qT, nt, 0, "pproj"))
            for a, b in ((4, 5), (6, 7)):
                fillers.extend(v_unit_chunks(a, "pproj", halves=(0,)))
                fillers.extend(v_unit_chunks(b, "pproj", halves=(0,)))
                fillers.extend(v_unit_chunks(a, "pproj", halves=(1,)))
                fillers.extend(v_unit_chunks(b, "pproj", halves=(1,)))
            fillers.extend(proj_unit_chunks(wk_sb, kT, 0, 1, "pproj"))
            fillers.extend(proj_unit_chunks(wq_sb, qT, 0, 1, "pproj"))

            # ---- attention qc0 (qw 0,1) ----
            for hp in range(NT):
                flush_deferred()
                av = {hh: pav.tile([65, 512], F32, tag="av", name=f"av0_{hp}_{hh}")
                      for hh in (0, 1)}
                if hp == 0:
                    attention(hp, 0, av, rate=0)
                    # V st2,3 needed from qw1 on; emit directly (pss free now)
                    for st in (2, 3):
                        for ch in v_unit_chunks(st, "pss"):
                            ch()
                    attention(hp, 1, av, rate=2)
                else:
                    attention(hp, 0, av, rate=2)
                    attention(hp, 1, av, rate=2)
                div_unit(hp, 0, av)

            # F2 fillers: remaining K/Q proj (sc=1) staggered ahead of their
            # consuming head pair, and output projection for s rows 0:512.
            f2 = []
            f2.append(proj_unit_chunks(wk_sb, kT, 1, 1, "pproj")
                      + proj_unit_chunks(wq_sb, qT, 1, 1, "pproj"))
            f2.append(proj_unit_chunks(wk_sb, kT, 2, 1, "pproj")
                      + proj_unit_chunks(wq_sb, qT, 2, 1, "pproj"))
            f2.append(ste_unit_chunks(0, "pproj", halves=(0,))
                      + ste_unit_chunks(1, "pproj", halves=(0,))
                      + ste_unit_chunks(0, "pproj", halves=(1,))
                      + ste_unit_chunks(1, "pproj", halves=(1,)))
            f2.append(proj_unit_chunks(wk_sb, kT, 3, 1, "pproj")
                      + proj_unit_chunks(wq_sb, qT, 3, 1, "pproj"))
            f2.append(ste_unit_chunks(2, "pproj", halves=(0,))
                      + ste_unit_chunks(3, "pproj", halves=(0,))
                      + ste_unit_chunks(2, "pproj", halves=(1,))
                      + ste_unit_chunks(3, "pproj", halves=(1,)))
            f2.append(proj_unit_chunks(wk_sb, kT, 4, 1, "pproj")
                      + proj_unit_chunks(wq_sb, qT, 4, 1, "pproj"))
            f2.append(proj_unit_chunks(wk_sb, kT, 5, 1, "pproj")
                      + proj_unit_chunks(wq_sb, qT, 5, 1, "pproj"))
            for grp in f2:
                fillers.extend(grp)

            # ---- attention qc1 (qw 2,3) ----
            for hp in range(NT):
                flush_deferred()
                av = {hh: pav.tile([65, 512], F32, tag="av", name=f"av1_{hp}_{hh}")
                      for hh in (0, 1)}
                attention(hp, 2, av, rate=1)
                attention(hp, 3, av, rate=1)
                div_unit(hp, 1, av)

            drain_fillers()
            flush_deferred()

            # ---- tail: output projection for s rows 512:1024 ----
            for st in range(4, ST):
                for ch in ste_unit_chunks(st, "pss"):
                    ch()

    nc.compile()
    return nc


def _get_compiled():
    global _compiled
    if _compiled is None:
        _compiled = _build_nc()
    return _compiled


def _prep_in_maps(x, W_attn, W_proj):
    import ml_dtypes

    BF = ml_dtypes.bfloat16
    x = np.asarray(x, dtype=np.float32)
    W_attn = np.asarray(W_attn, dtype=np.float32)
    W_proj = np.asarray(W_proj, dtype=np.float32)

    def wlayout(w):
        return np.ascontiguousarray(
            w.reshape(NT, 128, D).transpose(1, 0, 2)).astype(BF)

    wq = wlayout(np.ascontiguousarray(W_attn[:, 0:D]) * np.float32(0.125))
    wk = wlayout(np.ascontiguousarray(W_attn[:, D:2 * D]))
    wv = wlayout(np.ascontiguousarray(W_attn[:, 2 * D:3 * D]))
    wp = wlayout(W_proj)
    masks = _build_mask()

    xT = np.transpose(x, (0, 2, 1)).reshape(B, NT, 128, S).transpose(0, 2, 1, 3)
    xT = np.ascontiguousarray(xT).astype(BF)

    return [
        {"xT": xT[b], "wq": wq, "wk": wk, "wv": wv, "wp": wp, "masks": masks}
        for b in range(B)
    ]


def kernel(x, W_attn, W_proj):
    from concourse.bass_utils import run_bass_kernel_spmd

    nc = _get_compiled()
    in_maps = _prep_in_maps(x, W_attn, W_proj)
    res = run_bass_kernel_spmd(nc, in_maps, list(range(NC_)))
    y = np.stack([res.results[b]["y"] for b in range(B)], axis=0)
    return y.astype(np.float32)
